# revision 7
# baseline (speedup 1.0000x reference)
"""Mixtral BlockSparseTop2MLP with 2-bit HQQ weights on 8 Trainium2 NeuronCores.

v2 design (vs baseline):
  - Strided-group ffn sharding: core r owns groups g in [28r, 28r+28)
    (rows n with n mod 224 in that range).  Scales/zeros are sliced 8x
    instead of replicated; the zero-point correction needs a single
    28-partition indicator matmul per psum tile.
  - All operands host-pretransposed to k-major layouts: every DMA is a
    plain (non-xbar) copy.  s*z products, indicator matrices computed on
    host.
  - Local n order is g-major: n_local = g_loc*64 + q.  Gate/up psum is
    split 8 banks (g_loc 0..15) + 6 banks (g_loc 16..27).
  - Dequant: per 4-kt visit, 4 plane extracts (tensor_scalar shift+and,
    u16) + 4 plane mults (3D-AP tensor_tensor, scale broadcast over q).
  - Down-proj h @ w2^T runs in fp8 (e4m3) with DoubleRow perf mode:
    h8 = silu(gate)*up cast to fp8, w2 dequantized to fp8 with a 64x
    scale (descaled in the final psum->out copy).  Verified numerically:
    adds ~2e-3 rel error on top of the bf16 pipeline's 2.4e-3.
  - Output partials in bf16 [hid, m]; host sums the 8 partials in f32.
"""
import sys
import os
import json

sys.path.insert(0, "/opt/trn_rl_repo")

import numpy as np
import ml_dtypes

H = 4096          # hidden
F = 14336         # ffn
M = 512           # tokens
GS = 64
G1 = 224          # ffn-side groups (n % 224)
G2 = 64           # hidden-side groups (hid % 64)
NCORES = 8
GL = G1 // NCORES     # 28 local groups per core
Q = F // G1           # 64 rows per group
NSH = GL * Q          # 1792 ffn rows per core
KT = H // 128         # 32 k tiles
KC = 4                # kt per dequant visit
NV = KT // KC         # 8 visits per half-sweep
FT = NSH // 128       # 14 f tiles per core
NTA = 8               # half A n-tiles (g_loc 0..15)
NTB = 6               # half B n-tiles (g_loc 16..27)
JA = NTA * 128 // 4   # 256 packed cols in half A
JB = NTB * 128 // 4   # 192 packed cols in half B
J = JA + JB           # 448 packed cols per core

BF16 = ml_dtypes.bfloat16

LAST_EXEC_NS = None

_cache = {}


# ---------------------------------------------------------------------------
# walrus workaround: split multi-wait/multi-update sync info onto
# single-wait EventSemaphore carrier instructions at the BIR-JSON level.
# ---------------------------------------------------------------------------
def _carrier(engine, debug, name, wait=None, update=None):
    si = {"on_update": [update] if update else [], "on_wait": [wait] if wait else []}
    return {"debug": debug, "engine": engine, "ins": [], "name": name,
            "opcode": "EventSemaphore", "outs": [], "sync_info": si}


def _apply_multiwait_fix(nc):
    d = json.loads(nc.to_json_bytes())
    for fn in d.get("functions", []):
        for blk in fn.get("blocks", []):
            out = []
            for inst in blk.get("instructions", []):
                si = inst.get("sync_info")
                waits = (si or {}).get("on_wait", [])
                updates = (si or {}).get("on_update", [])
                post = []
                if si and len(waits) > 1:
                    for k, w in enumerate(waits[:-1]):
                        out.append(_carrier(inst["engine"], inst.get("debug", 0),
                                            f"{inst['name']}-xw{k}", wait=w))
                    si["on_wait"] = [waits[-1]]
                if si and len(updates) > 1:
                    for k, u in enumerate(updates[1:]):
                        post.append(_carrier(inst["engine"], inst.get("debug", 0),
                                             f"{inst['name']}-xu{k}", update=u))
                    si["on_update"] = updates[:1]
                out.append(inst)
                out.extend(post)
            blk["instructions"] = out
    fixed = json.dumps(d).encode()
    nc.to_json_bytes = lambda: fixed


# ---------------------------------------------------------------------------
# device program (identical on all 8 cores; per-core data differs only)
# ---------------------------------------------------------------------------
def _build(debug=False):
    import concourse.bass as bass
    import concourse.mybir as mybir
    import concourse.tile as tile

    AluOp = mybir.AluOpType
    Act = mybir.ActivationFunctionType
    DR = mybir.MatmulPerfMode.DoubleRow
    bf = mybir.dt.bfloat16
    u16 = mybir.dt.uint16
    f32 = mybir.dt.float32
    f8 = mybir.dt.float8e4

    nc = bass.Bass()

    xT_p = nc.declare_dram_parameter("xT", [H, M], bf, isOutput=False)
    pk1_p = nc.declare_dram_parameter("pk1", [H, J], u16, isOutput=False)
    pk3_p = nc.declare_dram_parameter("pk3", [H, J], u16, isOutput=False)
    pk2_p = nc.declare_dram_parameter("pk2", [NSH, H // 4], u16, isOutput=False)
    s1x_p = nc.declare_dram_parameter("s1x", [128, NSH], bf, isOutput=False)
    s3x_p = nc.declare_dram_parameter("s3x", [128, NSH], bf, isOutput=False)
    szT_p = nc.declare_dram_parameter("szT", [H, 64], bf, isOutput=False)
    s2d_p = nc.declare_dram_parameter("s2d", [NSH, G2], bf, isOutput=False)
    sz2T_p = nc.declare_dram_parameter("sz2T", [NSH, G2], bf, isOutput=False)
    ind1_p = nc.declare_dram_parameter("ind1", [64, NSH], bf, isOutput=False)
    ind2_p = nc.declare_dram_parameter("ind2", [G2, 512], bf, isOutput=False)
    out_p = nc.declare_dram_parameter("out", [H, M], bf, isOutput=True)
    if debug:
        dbg_gh = nc.declare_dram_parameter("dbg_gh", [128, FT * M], bf,
                                           isOutput=True)
        dbg_h8 = nc.declare_dram_parameter("dbg_h8", [128, FT * M], f8,
                                           isOutput=True)
        dbg_v2 = nc.declare_dram_parameter("dbg_v2", [128, FT * H], f8,
                                           isOutput=True)
        dbg_c = nc.declare_dram_parameter("dbg_c", [64, 2 * M], bf,
                                          isOutput=True)

    def ap(t, off, dims):
        return bass.AP(t.tensor, t.offset + off, [list(t.ap[0])] + dims)

    def dram_ap(p, off, dims):
        a = p[:, :]
        return bass.AP(a.tensor, off, dims)

    with tile.TileContext(nc) as tc:
        with (
            tc.tile_pool(name="const", bufs=1) as constp,
            tc.tile_pool(name="pk", bufs=4) as pkp,
            tc.tile_pool(name="tmp", bufs=2) as tmpp,
            tc.tile_pool(name="wh", bufs=3) as whp,
            tc.tile_pool(name="q2", bufs=3) as q2p,
            tc.tile_pool(name="v2b", bufs=2) as v2bp,
            tc.tile_pool(name="big", bufs=1) as bigp,
            tc.tile_pool(name="cst", bufs=1) as cstp,
            tc.tile_pool(name="ob", bufs=3) as obp,
            tc.tile_pool(name="ps", bufs=8, space="PSUM") as psp,
        ):
            # ---- resident inputs (DMA order = consumption order) ----------
            szT = constp.tile([128, KT, 64], bf, name="szT")
            nc.sync.dma_start(
                szT[:], dram_ap(szT_p, 0,
                                [[64, 128], [64 * 128, KT], [1, 64]]))
            xT = constp.tile([128, KT, M], bf, name="xT")

            def xt_dma(c):
                nc.sync.dma_start(
                    xT[:, 4 * c:4 * c + 4, :],
                    dram_ap(xT_p, 4 * c * 128 * M,
                            [[M, 128], [128 * M, 4], [1, M]]))

            xt_dma(0)
            pk_pre0 = {}
            _pk = pkp.tile([128, KC, JA], u16, name="pk", tag="pk")
            nc.sync.dma_start(_pk[:], dram_ap(pk1_p, 0,
                                              [[J, 128], [J * 128, KC], [1, JA]]))
            pk_pre0[("pk1", 0, 0)] = _pk
            s1x = constp.tile([128, NSH], bf, name="s1x")
            nc.sync.dma_start(s1x[:], s1x_p[:, :])
            _pk = pkp.tile([128, KC, JA], u16, name="pk", tag="pk")
            nc.sync.dma_start(_pk[:], dram_ap(pk1_p, (KC * 128) * J,
                                              [[J, 128], [J * 128, KC], [1, JA]]))
            pk_pre0[("pk1", 0, 1)] = _pk
            for c in range(1, KT // 4):
                xt_dma(c)
            ind1 = constp.tile([64, NSH], bf, name="ind1")
            nc.sync.dma_start(ind1[:], ind1_p[:, :])
            s2d = constp.tile([128, FT, G2], bf, name="s2d")
            nc.sync.dma_start(
                s2d[:], dram_ap(s2d_p, 0,
                                [[G2, 128], [G2 * 128, FT], [1, G2]]))
            s3x = constp.tile([128, NSH], bf, name="s3x")
            sz2T = constp.tile([128, FT, G2], bf, name="sz2T")
            ind2 = constp.tile([G2, 512], bf, name="ind2")

            def late_dmas():
                nc.sync.dma_start(s3x[:], s3x_p[:, :])
                nc.sync.dma_start(
                    sz2T[:], dram_ap(sz2T_p, 0,
                                     [[G2, 128], [G2 * 128, FT], [1, G2]]))
                nc.sync.dma_start(ind2[:], ind2_p[:, :])

            gh = bigp.tile([128, FT, M], bf, name="gh")
            h8 = bigp.tile([128, FT, M], f8, name="h8")
            v2 = bigp.tile([128, FT // 2, 2, H], f8, name="v2")
            c_sb = {}

            # ---- zero-point corrections C1/C3 = (s*z) @ x^T ----------------
            pc1 = psp.tile([64, M], f32, name="pc1", tag="acc")
            for kt in range(KT):
                nc.tensor.matmul(pc1[:], szT[:, kt, :], xT[:, kt, :],
                                 start=(kt == 0), stop=(kt == KT - 1))
            csb = cstp.tile([64, M], bf, name="csb")
            nc.scalar.copy(csb[:], pc1[:])
            c_sb[1] = csb[0:GL, :]
            c_sb[3] = csb[32:32 + GL, :]

            # ---- w2 dequant blocks (interleaved into gate/up sweeps) -------
            def w2_block(ft):
                """hid' (plane-major) space: hid' = i*1024 + j2, hid = 4*j2+i.
                v2b[:, hid'] = tmp2[:, i, j2] * s2d[:, ft, i*16 + j2%16]."""
                q2 = q2p.tile([128, H // 4], u16, name="q2", tag="q2")
                nc.sync.dma_start(q2[:], pk2_p[ft * 128:(ft + 1) * 128, :])
                tmp2 = tmpp.tile([128, 4, H // 4], u16, name="tmp2", tag="tmp")
                v2b = v2bp.tile([128, H], bf, name="v2b", tag="v2b")
                for i in range(4):
                    nc.vector.tensor_scalar(
                        out=ap(tmp2[:], i * (H // 4), [[1, H // 4]]),
                        in0=q2[:], scalar1=(3 - i) * 2, scalar2=3,
                        op0=AluOp.logical_shift_right, op1=AluOp.bitwise_and)
                nc.vector.tensor_tensor(
                    out=ap(v2b[:], 0, [[1024, 4], [16, 64], [1, 16]]),
                    in0=ap(tmp2[:], 0, [[1024, 4], [16, 64], [1, 16]]),
                    in1=ap(s2d[:, ft, :], 0, [[16, 4], [0, 64], [1, 16]]),
                    op=AluOp.mult)
                nc.scalar.copy(v2[:, ft // 2, ft % 2, :], v2b[:])

            w2_left = list(range(FT))
            visit_no = [0]
            pk_pre = dict(pk_pre0)

            def pk_dma(pk_p, key, v, jlo, jw):
                pk = pkp.tile([128, KC, jw], u16, name="pk", tag="pk")
                nc.sync.dma_start(
                    pk[:], dram_ap(pk_p, (v * KC * 128) * J + jlo,
                                   [[J, 128], [J * 128, KC], [1, jw]]))
                return pk

            def tick_w2():
                visit_no[0] += 1
                if visit_no[0] == 12:
                    late_dmas()
                if visit_no[0] in (15, 16):
                    v = visit_no[0] - 15
                    pk_pre[("pk3", 0, v)] = pk_dma(pk3_p, "pk3", v, 0, JA)
                if visit_no[0] % 2 == 0 and w2_left:
                    w2_block(w2_left.pop(0))

            # ---- gate / up: extract + scale + matmul ----------------------
            def wmatmul_phase(pk_p, sx, w, pname, tile_done):
                ps_all = []
                for half, (nt0, ntn, jlo, jw) in enumerate(
                        ((0, NTA, 0, JA), (NTA, NTB, JA, JB))):
                    ng = ntn * 128          # n rows in this half
                    njg = jw // Q           # jg count (4 or 3)
                    nm = KC * njg           # merged (kt, jg) count (16 or 12)
                    sxbase = 0 if half == 0 else NV * 4 * KC * (JA // Q) * 2
                    pg = [psp.tile([128, M], f32, name=f"p{w}_{half}_{k}",
                                   tag="acc") for k in range(ntn)]
                    for v in range(NV):
                        pk = pk_pre.pop((pname, half, v), None)
                        if pk is None:
                            pk = pk_dma(pk_p, pname, v, jlo, jw)
                        tmp = tmpp.tile([128, 4, KC * jw], u16, name="tmp",
                                        tag="tmp")
                        wh = whp.tile([128, KC, ng], bf, name="wh", tag="wh")
                        for i in range(4):
                            nc.vector.tensor_scalar(
                                out=ap(tmp[:], i * (KC * jw),
                                       [[jw, KC], [1, jw]]),
                                in0=pk[:], scalar1=(3 - i) * 2, scalar2=3,
                                op0=AluOp.logical_shift_right,
                                op1=AluOp.bitwise_and)
                            # wh[:, dk, 256*jg + 64*i + q] =
                            #   tmp[:, i, dk*jw + 64*jg + q] * s[4jg+i, kt]
                            # merged (dk, jg) dim + duplicated-scale pairs to
                            # hit the DVE 2x mode (innermost [1, 2]).
                            nc.vector.tensor_tensor(
                                out=ap(wh[:], 64 * i,
                                       [[256, nm], [2, Q // 2], [1, 2]]),
                                in0=ap(tmp[:], i * (KC * jw),
                                       [[Q, nm], [2, Q // 2], [1, 2]]),
                                in1=ap(sx[:], sxbase + v * (KC * njg * 4 * 2)
                                       + i * (njg * KC * 2),
                                       [[2, nm], [0, Q // 2], [1, 2]]),
                                op=AluOp.mult)
                        for dk in range(KC):
                            kt = v * KC + dk
                            for k in range(ntn):
                                nc.tensor.matmul(
                                    pg[k][:],
                                    wh[:, dk, k * 128:(k + 1) * 128],
                                    xT[:, kt, :],
                                    start=(kt == 0), stop=False)
                        tick_w2()
                    rlo = 0 if w == 1 else 32
                    for k in range(ntn):
                        base = (nt0 + k) * 128
                        nc.tensor.matmul(pg[k][:],
                                         ind1[rlo:rlo + GL, base:base + 128],
                                         c_sb[w][:], start=False, stop=True)
                        tile_done(nt0 + k, pg[k])
                    ps_all.extend(pg)
                return ps_all

            # gate: psum -> silu -> gh (bf16), per tile as corrections land
            def gate_done(k, pg):
                nc.scalar.activation(gh[:, k, :], pg[:], Act.Silu)

            wmatmul_phase(pk1_p, s1x, 1, "pk1", gate_done)

            # up: h8 = silu(gate) * up (fp8), per tile
            def up_done(k, pg):
                nc.vector.tensor_tensor(out=h8[:, k, :], in0=pg[:],
                                        in1=gh[:, k, :], op=AluOp.mult)

            wmatmul_phase(pk3_p, s3x, 3, "pk3", up_done)
            # any w2 dequant blocks not yet emitted
            while w2_left:
                w2_block(w2_left.pop(0))

            # ---- w2 correction C2 = (s2*z2*64) @ h^T ----------------------
            pc2 = psp.tile([G2, M], f32, name="pc2", tag="acc")
            for ft in range(FT):
                nc.tensor.matmul(pc2[:], sz2T[:, ft, :], h8[:, ft, :],
                                 start=(ft == 0), stop=(ft == FT - 1))
            c2 = cstp.tile([G2, M], bf, name="c2")
            nc.scalar.copy(c2[:], pc2[:])

            if debug:
                nc.sync.dma_start(dbg_gh[:, :], gh[:])
                nc.sync.dma_start(dbg_h8[:, :], h8[:])
                nc.sync.dma_start(dbg_v2[:, :], v2[:])
                nc.sync.dma_start(dbg_c[:, 0:M], csb[:])
                nc.sync.dma_start(dbg_c[:, M:2 * M], c2[:])

            # ---- out^T = (v2 . h8)/64 - C2, fp8 DoubleRow over f-pairs ----
            for hg in range(4):
                po = [psp.tile([128, M], f32, name=f"po{hg}_{k}", tag="acc")
                      for k in range(8)]
                for t in range(FT // 2):
                    for k in range(8):
                        ht = hg * 8 + k
                        nc.tensor.matmul(
                            po[k][:],
                            v2[:, t, :, ht * 128:(ht + 1) * 128],
                            h8[:, 2 * t:2 * t + 2, :],
                            start=(t == 0), stop=False, perf_mode=DR)
                for k in range(8):
                    ht = hg * 8 + k
                    nc.tensor.matmul(
                        po[k][:], ind2[:, hg * 128:(hg + 1) * 128], c2[:],
                        start=False, stop=True)
                    ob = obp.tile([128, M], bf, name="ob", tag="ob")
                    nc.scalar.activation(ob[:], po[k][:], Act.Copy,
                                         scale=1.0 / 64.0)
                    nc.sync.dma_start(out_p[ht * 128:(ht + 1) * 128, :], ob[:])
    return nc


def _get_nc():
    if "nc" not in _cache:
        nc = _build()
        _apply_multiwait_fix(nc)
        _cache["nc"] = nc
    return _cache["nc"]


def build_in_maps(inp):
    f32 = np.float32
    x_bf = np.asarray(inp["x"], dtype=f32).astype(BF16)
    xT = np.ascontiguousarray(x_bf.T)                       # (4096, 512)
    qw1_u = np.asarray(inp["qw1"]).astype(np.uint16)        # (3584, 4096)
    qw3_u = np.asarray(inp["qw3"]).astype(np.uint16)
    qw2_u = np.asarray(inp["qw2"]).astype(np.uint16)        # (1024, 14336)
    s1_bf = np.asarray(inp["s1"], dtype=f32).astype(BF16)   # (224, 4096)
    z1_bf = np.asarray(inp["z1"], dtype=f32).astype(BF16)
    s3_bf = np.asarray(inp["s3"], dtype=f32).astype(BF16)
    z3_bf = np.asarray(inp["z3"], dtype=f32).astype(BF16)
    s2_bf = (np.asarray(inp["s2"], dtype=f32).astype(BF16).astype(f32)
             * 64.0).astype(BF16)                           # (64, 14336), x64
    z2_bf = np.asarray(inp["z2"], dtype=f32).astype(BF16)
    sz1 = (s1_bf.astype(f32) * z1_bf.astype(f32)).astype(BF16)
    sz3 = (s3_bf.astype(f32) * z3_bf.astype(f32)).astype(BF16)
    sz2 = (s2_bf.astype(f32) * z2_bf.astype(f32)).astype(BF16)

    def szp(a, b):
        z = np.zeros((64, H), dtype=BF16)
        z[0:GL] = a
        z[32:32 + GL] = b
        return z.T  # (4096, 64)

    jg_i, q_i = np.meshgrid(np.arange(7), np.arange(Q), indexing="ij")
    gl_i, q2_i = np.meshgrid(np.arange(GL), np.arange(Q), indexing="ij")

    def sx(sbf):
        # sbf: (28, 4096) local-group scales. Layout per partition p:
        # half A: [v:8][i:4][m=4*dk+jg:16][d:2]; half B (+1024): njg=3.
        out = np.zeros((128, NSH), dtype=BF16)
        for half, (njg, jg0, base) in enumerate(((4, 0, 0), (3, 4, 1024))):
            v, i, m, dd, p = np.meshgrid(
                np.arange(NV), np.arange(4), np.arange(KC * njg), np.arange(2),
                np.arange(128), indexing="ij")
            dk = m // njg
            jg = m % njg
            g = 4 * (jg0 + jg) + i
            k = (v * KC + dk) * 128 + p
            idx = base + v * (4 * KC * njg * 2) + i * (njg * KC * 2) + m * 2 + dd
            out[p.reshape(-1), idx.reshape(-1)] = sbf[g.reshape(-1), k.reshape(-1)]
        return out

    # s2 column permutation: newcol i*16+l <- oldcol 4l+i
    i_, l_ = np.meshgrid(np.arange(4), np.arange(16), indexing="ij")
    s2perm = (4 * l_ + i_).reshape(-1)

    n_loc = np.arange(NSH)
    ind1g = np.where(n_loc[None, :] // Q == np.arange(GL)[:, None],
                     -1.0, 0.0).astype(BF16)
    ind1 = np.zeros((64, NSH), dtype=BF16)
    ind1[0:GL] = ind1g
    ind1[32:32 + GL] = ind1g
    i_c, c_c = np.meshgrid(np.arange(4), np.arange(128), indexing="ij")
    g2_of = ((4 * c_c + i_c) % 64).reshape(-1)      # (512,) for col i*128+c
    ind2 = np.where(g2_of[None, :] == np.arange(G2)[:, None],
                    -1.0, 0.0).astype(BF16)

    in_maps = []
    for r in range(NCORES):
        rows = (56 * q_i + 7 * r + jg_i).reshape(-1)        # j = jg*64+q
        fcols = (224 * q2_i + 28 * r + gl_i).reshape(-1)    # n_loc = gl*64+q
        gs = slice(GL * r, GL * (r + 1))
        in_maps.append({
            "xT": xT,
            "pk1": np.ascontiguousarray(qw1_u[rows].T),     # (4096, 448)
            "pk3": np.ascontiguousarray(qw3_u[rows].T),
            "pk2": np.ascontiguousarray(qw2_u[:, fcols].T),  # (1792, 1024)
            "s1x": sx(s1_bf[gs]),
            "s3x": sx(s3_bf[gs]),
            "szT": np.ascontiguousarray(szp(sz1[gs], sz3[gs])),
            "s2d": np.ascontiguousarray(s2_bf[:, fcols].T[:, s2perm]),
            "sz2T": np.ascontiguousarray(sz2[:, fcols].T),
            "ind1": ind1,
            "ind2": ind2,
        })
    return in_maps


def kernel(x, qw1, s1, z1, qw3, s3, z3, qw2, s2, z2, groupsize=64, **_ignored):
    from concourse.bass_utils import run_bass_kernel_spmd

    global LAST_EXEC_NS

    in_maps = build_in_maps(dict(x=x, qw1=qw1, s1=s1, z1=z1, qw3=qw3, s3=s3,
                                 z3=z3, qw2=qw2, s2=s2, z2=z2))
    _cache["in_maps"] = in_maps

    nc = _get_nc()
    trace = bool(os.environ.get("BASS_HQQ_TRACE"))
    try:
        res = run_bass_kernel_spmd(nc, in_maps, list(range(NCORES)), trace=trace)
    except ModuleNotFoundError:
        res = run_bass_kernel_spmd(nc, in_maps, list(range(NCORES)), trace=False)
    LAST_EXEC_NS = res.exec_time_ns

    acc = np.zeros((H, M), dtype=np.float32)
    for r in range(NCORES):
        acc += np.asarray(res.results[r]["out"], dtype=np.float32)
    hidp = np.arange(H)
    hid = 4 * (hidp % 1024) + hidp // 1024
    out = np.empty((H, M), dtype=np.float32)
    out[hid] = acc
    return out.T.astype(np.float32)


# revision 14
# speedup vs baseline: 1.1597x; 1.1597x over previous
"""Mixtral BlockSparseTop2MLP with 2-bit HQQ weights on 8 Trainium2 NeuronCores.

v2 design (vs baseline):
  - Strided-group ffn sharding: core r owns groups g in [28r, 28r+28)
    (rows n with n mod 224 in that range).  Scales/zeros are sliced 8x
    instead of replicated; the zero-point correction needs a single
    28-partition indicator matmul per psum tile.
  - All operands host-pretransposed to k-major layouts: every DMA is a
    plain (non-xbar) copy.  s*z products, indicator matrices computed on
    host.
  - Local n order is g-major: n_local = g_loc*64 + q.  Gate/up psum is
    split 8 banks (g_loc 0..15) + 6 banks (g_loc 16..27).
  - Dequant: per 4-kt visit, 4 plane extracts (tensor_scalar shift+and,
    u16) + 4 plane mults (3D-AP tensor_tensor, scale broadcast over q).
  - Down-proj h @ w2^T runs in fp8 (e4m3) with DoubleRow perf mode:
    h8 = silu(gate)*up cast to fp8, w2 dequantized to fp8 with a 64x
    scale (descaled in the final psum->out copy).  Verified numerically:
    adds ~2e-3 rel error on top of the bf16 pipeline's 2.4e-3.
  - Output partials in bf16 [hid, m]; host sums the 8 partials in f32.
"""
import sys
import os
import json

sys.path.insert(0, "/opt/trn_rl_repo")

import numpy as np
import ml_dtypes

H = 4096          # hidden
F = 14336         # ffn
M = 512           # tokens
GS = 64
G1 = 224          # ffn-side groups (n % 224)
G2 = 64           # hidden-side groups (hid % 64)
NCORES = 8
GL = G1 // NCORES     # 28 local groups per core
Q = F // G1           # 64 rows per group
NSH = GL * Q          # 1792 ffn rows per core
KT = H // 128         # 32 k tiles
KC = 4                # kt per dequant visit
NV = KT // KC         # 8 visits per half-sweep
FT = NSH // 128       # 14 f tiles per core
NTA = 8               # half A n-tiles (g_loc 0..15)
NTB = 6               # half B n-tiles (g_loc 16..27)
JA = NTA * 128 // 4   # 256 packed cols in half A
JB = NTB * 128 // 4   # 192 packed cols in half B
J = JA + JB           # 448 packed cols per core

BF16 = ml_dtypes.bfloat16

LAST_EXEC_NS = None

_cache = {}


# ---------------------------------------------------------------------------
# walrus workaround: split multi-wait/multi-update sync info onto
# single-wait EventSemaphore carrier instructions at the BIR-JSON level.
# ---------------------------------------------------------------------------
def _carrier(engine, debug, name, wait=None, update=None):
    si = {"on_update": [update] if update else [], "on_wait": [wait] if wait else []}
    return {"debug": debug, "engine": engine, "ins": [], "name": name,
            "opcode": "EventSemaphore", "outs": [], "sync_info": si}


def _apply_multiwait_fix(nc):
    d = json.loads(nc.to_json_bytes())
    for fn in d.get("functions", []):
        for blk in fn.get("blocks", []):
            out = []
            for inst in blk.get("instructions", []):
                si = inst.get("sync_info")
                waits = (si or {}).get("on_wait", [])
                updates = (si or {}).get("on_update", [])
                post = []
                if si and len(waits) > 1:
                    for k, w in enumerate(waits[:-1]):
                        out.append(_carrier(inst["engine"], inst.get("debug", 0),
                                            f"{inst['name']}-xw{k}", wait=w))
                    si["on_wait"] = [waits[-1]]
                if si and len(updates) > 1:
                    for k, u in enumerate(updates[1:]):
                        post.append(_carrier(inst["engine"], inst.get("debug", 0),
                                             f"{inst['name']}-xu{k}", update=u))
                    si["on_update"] = updates[:1]
                out.append(inst)
                out.extend(post)
            blk["instructions"] = out
    fixed = json.dumps(d).encode()
    nc.to_json_bytes = lambda: fixed


# ---------------------------------------------------------------------------
# device program (identical on all 8 cores; per-core data differs only)
# ---------------------------------------------------------------------------
def _build(debug=False):
    import concourse.bass as bass
    import concourse.mybir as mybir
    import concourse.tile as tile

    AluOp = mybir.AluOpType
    Act = mybir.ActivationFunctionType
    DR = mybir.MatmulPerfMode.DoubleRow
    bf = mybir.dt.bfloat16
    u16 = mybir.dt.uint16
    f32 = mybir.dt.float32
    f8 = mybir.dt.float8e4

    nc = bass.Bass()

    xT_p = nc.declare_dram_parameter("xT", [H, M], bf, isOutput=False)
    pk1_p = nc.declare_dram_parameter("pk1", [H, J], u16, isOutput=False)
    pk3_p = nc.declare_dram_parameter("pk3", [H, J], u16, isOutput=False)
    pk2_p = nc.declare_dram_parameter("pk2", [NSH, H // 4], u16, isOutput=False)
    s1x_p = nc.declare_dram_parameter("s1x", [128, NSH], bf, isOutput=False)
    s3x_p = nc.declare_dram_parameter("s3x", [128, NSH], bf, isOutput=False)
    szT_p = nc.declare_dram_parameter("szT", [H, 64], bf, isOutput=False)
    s2d_p = nc.declare_dram_parameter("s2d", [NSH, G2], bf, isOutput=False)
    sz2T_p = nc.declare_dram_parameter("sz2T", [NSH, G2], bf, isOutput=False)
    ind1_p = nc.declare_dram_parameter("ind1", [64, NSH], bf, isOutput=False)
    ind2_p = nc.declare_dram_parameter("ind2", [G2, 512], bf, isOutput=False)
    out_p = nc.declare_dram_parameter("out", [H, M], bf, isOutput=True)
    if debug:
        dbg_gh = nc.declare_dram_parameter("dbg_gh", [128, FT * M], bf,
                                           isOutput=True)
        dbg_h8 = nc.declare_dram_parameter("dbg_h8", [128, FT * M], f8,
                                           isOutput=True)
        dbg_v2 = nc.declare_dram_parameter("dbg_v2", [128, FT * H], f8,
                                           isOutput=True)
        dbg_c = nc.declare_dram_parameter("dbg_c", [64, 2 * M], bf,
                                          isOutput=True)

    def ap(t, off, dims):
        return bass.AP(t.tensor, t.offset + off, [list(t.ap[0])] + dims)

    def dram_ap(p, off, dims):
        a = p[:, :]
        return bass.AP(a.tensor, off, dims)

    with tile.TileContext(nc) as tc:
        with (
            tc.tile_pool(name="const", bufs=1) as constp,
            tc.tile_pool(name="pk", bufs=4) as pkp,
            tc.tile_pool(name="tmp", bufs=2) as tmpp,
            tc.tile_pool(name="wh", bufs=3) as whp,
            tc.tile_pool(name="q2", bufs=3) as q2p,
            tc.tile_pool(name="v2b", bufs=2) as v2bp,
            tc.tile_pool(name="big", bufs=1) as bigp,
            tc.tile_pool(name="cst", bufs=1) as cstp,
            tc.tile_pool(name="ob", bufs=3) as obp,
            tc.tile_pool(name="ps", bufs=8, space="PSUM") as psp,
        ):
            # ---- resident inputs (DMA order = consumption order) ----------
            szT = constp.tile([128, KT, 64], bf, name="szT")
            nc.sync.dma_start(
                szT[:], dram_ap(szT_p, 0,
                                [[64, 128], [64 * 128, KT], [1, 64]]))
            xT = constp.tile([128, KT, M], bf, name="xT")

            def xt_dma(c):
                nc.sync.dma_start(
                    xT[:, 4 * c:4 * c + 4, :],
                    dram_ap(xT_p, 4 * c * 128 * M,
                            [[M, 128], [128 * M, 4], [1, M]]))

            xt_dma(0)
            pk_pre0 = {}
            _pk = pkp.tile([128, KC, JA], u16, name="pk", tag="pk")
            nc.sync.dma_start(_pk[:], dram_ap(pk1_p, 0,
                                              [[J, 128], [J * 128, KC], [1, JA]]))
            pk_pre0[("pk1", 0, 0)] = _pk
            s1x = constp.tile([128, NSH], bf, name="s1x")
            nc.sync.dma_start(s1x[:], s1x_p[:, :])
            _pk = pkp.tile([128, KC, JA], u16, name="pk", tag="pk")
            nc.sync.dma_start(_pk[:], dram_ap(pk1_p, (KC * 128) * J,
                                              [[J, 128], [J * 128, KC], [1, JA]]))
            pk_pre0[("pk1", 0, 1)] = _pk
            for c in range(1, KT // 4):
                xt_dma(c)
            ind1 = constp.tile([64, NSH], bf, name="ind1")
            nc.sync.dma_start(ind1[:], ind1_p[:, :])
            s2d = constp.tile([128, FT, G2], bf, name="s2d")
            nc.sync.dma_start(
                s2d[:], dram_ap(s2d_p, 0,
                                [[G2, 128], [G2 * 128, FT], [1, G2]]))
            s3x = constp.tile([128, NSH], bf, name="s3x")
            sz2T = constp.tile([128, FT, G2], bf, name="sz2T")
            ind2 = constp.tile([G2, 512], bf, name="ind2")

            def late_dmas():
                nc.sync.dma_start(s3x[:], s3x_p[:, :])
                nc.sync.dma_start(
                    sz2T[:], dram_ap(sz2T_p, 0,
                                     [[G2, 128], [G2 * 128, FT], [1, G2]]))
                nc.sync.dma_start(ind2[:], ind2_p[:, :])

            gh = bigp.tile([128, FT, M], bf, name="gh")
            h8 = bigp.tile([128, FT, M], f8, name="h8")
            v2 = bigp.tile([128, FT // 2, 2, H], f8, name="v2")
            c_sb = {}

            # ---- zero-point corrections C1/C3 = (s*z) @ x^T ----------------
            pc1 = psp.tile([64, M], f32, name="pc1", tag="acc")
            for kt in range(KT):
                nc.tensor.matmul(pc1[:], szT[:, kt, :], xT[:, kt, :],
                                 start=(kt == 0), stop=(kt == KT - 1))
            csb = cstp.tile([64, M], bf, name="csb")
            nc.scalar.copy(csb[:], pc1[:])
            c_sb[1] = csb[0:GL, :]
            c_sb[3] = csb[32:32 + GL, :]

            # ---- w2 dequant blocks (interleaved into gate/up sweeps) -------
            def w2_block(ft):
                """hid' (plane-major) space: hid' = i*1024 + j2, hid = 4*j2+i.
                v2b[:, hid'] = tmp2[:, i, j2] * s2d[:, ft, i*16 + j2%16]."""
                q2 = q2p.tile([128, H // 4], u16, name="q2", tag="q2")
                nc.sync.dma_start(q2[:], pk2_p[ft * 128:(ft + 1) * 128, :])
                tmp2 = tmpp.tile([128, 4, H // 4], u16, name="tmp2", tag="tmp")
                v2b = v2bp.tile([128, H], bf, name="v2b", tag="v2b")
                for i in range(4):
                    nc.vector.tensor_scalar(
                        out=ap(tmp2[:], i * (H // 4), [[1, H // 4]]),
                        in0=q2[:], scalar1=(3 - i) * 2, scalar2=3,
                        op0=AluOp.logical_shift_right, op1=AluOp.bitwise_and)
                nc.vector.tensor_tensor(
                    out=ap(v2b[:], 0, [[1024, 4], [16, 64], [1, 16]]),
                    in0=ap(tmp2[:], 0, [[1024, 4], [16, 64], [1, 16]]),
                    in1=ap(s2d[:, ft, :], 0, [[16, 4], [0, 64], [1, 16]]),
                    op=AluOp.mult)
                nc.scalar.copy(v2[:, ft // 2, ft % 2, :], v2b[:])

            w2_left = list(range(FT))
            visit_no = [0]
            pk_pre = dict(pk_pre0)

            def pk_dma(pk_p, key, v, jlo, jw):
                pk = pkp.tile([128, KC, jw], u16, name="pk", tag="pk")
                nc.sync.dma_start(
                    pk[:], dram_ap(pk_p, (v * KC * 128) * J + jlo,
                                   [[J, 128], [J * 128, KC], [1, jw]]))
                return pk

            def tick_w2():
                visit_no[0] += 1
                if visit_no[0] == 12:
                    late_dmas()
                if visit_no[0] in (15, 16):
                    v = visit_no[0] - 15
                    pk_pre[("pk3", 0, v)] = pk_dma(pk3_p, "pk3", v, 0, JA)
                if visit_no[0] % 2 == 0 and w2_left:
                    w2_block(w2_left.pop(0))

            # ---- gate / up: extract + scale + matmul ----------------------
            def wmatmul_phase(pk_p, sx, w, pname, tile_done):
                ps_all = []
                for half, (nt0, ntn, jlo, jw) in enumerate(
                        ((0, NTA, 0, JA), (NTA, NTB, JA, JB))):
                    ng = ntn * 128          # n rows in this half
                    njg = jw // Q           # jg count (4 or 3)
                    nm = KC * njg           # merged (kt, jg) count (16 or 12)
                    sxbase = 0 if half == 0 else NV * 4 * KC * (JA // Q) * 2
                    pg = [psp.tile([128, M], f32, name=f"p{w}_{half}_{k}",
                                   tag="acc") for k in range(ntn)]
                    for v in range(NV):
                        pk = pk_pre.pop((pname, half, v), None)
                        if pk is None:
                            pk = pk_dma(pk_p, pname, v, jlo, jw)
                        tmp = tmpp.tile([128, 4, KC * jw], u16, name="tmp",
                                        tag="tmp")
                        wh = whp.tile([128, KC, ng], bf, name="wh", tag="wh")
                        for i in range(4):
                            nc.vector.tensor_scalar(
                                out=ap(tmp[:], i * (KC * jw),
                                       [[jw, KC], [1, jw]]),
                                in0=pk[:], scalar1=(3 - i) * 2, scalar2=3,
                                op0=AluOp.logical_shift_right,
                                op1=AluOp.bitwise_and)
                            # wh[:, dk, 256*jg + 64*i + q] =
                            #   tmp[:, i, dk*jw + 64*jg + q] * s[4jg+i, kt]
                            # merged (dk, jg) dim + duplicated-scale pairs to
                            # hit the DVE 2x mode (innermost [1, 2]).
                            nc.vector.tensor_tensor(
                                out=ap(wh[:], 64 * i,
                                       [[256, nm], [2, Q // 2], [1, 2]]),
                                in0=ap(tmp[:], i * (KC * jw),
                                       [[Q, nm], [2, Q // 2], [1, 2]]),
                                in1=ap(sx[:], sxbase + v * (KC * njg * 4 * 2)
                                       + i * (njg * KC * 2),
                                       [[2, nm], [0, Q // 2], [1, 2]]),
                                op=AluOp.mult)
                        for dk in range(KC):
                            kt = v * KC + dk
                            for k in range(ntn):
                                nc.tensor.matmul(
                                    pg[k][:],
                                    wh[:, dk, k * 128:(k + 1) * 128],
                                    xT[:, kt, :],
                                    start=(kt == 0), stop=False)
                        tick_w2()
                    rlo = 0 if w == 1 else 32
                    for k in range(ntn):
                        base = (nt0 + k) * 128
                        nc.tensor.matmul(pg[k][:],
                                         ind1[rlo:rlo + GL, base:base + 128],
                                         c_sb[w][:], start=False, stop=True)
                        tile_done(nt0 + k, pg[k])
                    ps_all.extend(pg)
                return ps_all

            # gate: psum -> silu -> gh (bf16), per tile as corrections land
            def gate_done(k, pg):
                nc.scalar.activation(gh[:, k, :], pg[:], Act.Silu)

            wmatmul_phase(pk1_p, s1x, 1, "pk1", gate_done)

            # up: h8 = silu(gate) * up (fp8), per tile
            def up_done(k, pg):
                nc.vector.tensor_tensor(out=h8[:, k, :], in0=pg[:],
                                        in1=gh[:, k, :], op=AluOp.mult)

            wmatmul_phase(pk3_p, s3x, 3, "pk3", up_done)
            # any w2 dequant blocks not yet emitted
            while w2_left:
                w2_block(w2_left.pop(0))

            # ---- w2 correction C2 = (s2*z2*64) @ h^T ----------------------
            pc2 = psp.tile([G2, M], f32, name="pc2", tag="acc")
            for ft in range(FT):
                nc.tensor.matmul(pc2[:], sz2T[:, ft, :], h8[:, ft, :],
                                 start=(ft == 0), stop=(ft == FT - 1))
            c2 = cstp.tile([G2, M], bf, name="c2")
            nc.scalar.copy(c2[:], pc2[:])

            if debug:
                nc.sync.dma_start(dbg_gh[:, :], gh[:])
                nc.sync.dma_start(dbg_h8[:, :], h8[:])
                nc.sync.dma_start(dbg_v2[:, :], v2[:])
                nc.sync.dma_start(dbg_c[:, 0:M], csb[:])
                nc.sync.dma_start(dbg_c[:, M:2 * M], c2[:])

            # ---- out^T = (v2 . h8)/64 - C2, fp8 DoubleRow over f-pairs ----
            for hg in range(4):
                po = [psp.tile([128, M], f32, name=f"po{hg}_{k}", tag="acc")
                      for k in range(8)]
                for t in range(FT // 2):
                    for k in range(8):
                        ht = hg * 8 + k
                        nc.tensor.matmul(
                            po[k][:],
                            v2[:, t, :, ht * 128:(ht + 1) * 128],
                            h8[:, 2 * t:2 * t + 2, :],
                            start=(t == 0), stop=False, perf_mode=DR)
                for k in range(8):
                    ht = hg * 8 + k
                    nc.tensor.matmul(
                        po[k][:], ind2[:, hg * 128:(hg + 1) * 128], c2[:],
                        start=False, stop=True)
                    ob = obp.tile([128, M], bf, name="ob", tag="ob")
                    nc.scalar.activation(ob[:], po[k][:], Act.Copy,
                                         scale=1.0 / 64.0)
                    nc.sync.dma_start(out_p[ht * 128:(ht + 1) * 128, :], ob[:])
    return nc


def _get_nc():
    if "nc" not in _cache:
        nc = _build()
        _apply_multiwait_fix(nc)
        _cache["nc"] = nc
    return _cache["nc"]


def build_in_maps(inp):
    f32 = np.float32
    x_bf = np.asarray(inp["x"], dtype=f32).astype(BF16)
    xT = np.ascontiguousarray(x_bf.T)                       # (4096, 512)
    qw1_u = np.asarray(inp["qw1"]).astype(np.uint16)        # (3584, 4096)
    qw3_u = np.asarray(inp["qw3"]).astype(np.uint16)
    qw2_u = np.asarray(inp["qw2"]).astype(np.uint16)        # (1024, 14336)
    s1_bf = np.asarray(inp["s1"], dtype=f32).astype(BF16)   # (224, 4096)
    z1_bf = np.asarray(inp["z1"], dtype=f32).astype(BF16)
    s3_bf = np.asarray(inp["s3"], dtype=f32).astype(BF16)
    z3_bf = np.asarray(inp["z3"], dtype=f32).astype(BF16)
    s2_bf = (np.asarray(inp["s2"], dtype=f32).astype(BF16).astype(f32)
             * 64.0).astype(BF16)                           # (64, 14336), x64
    z2_bf = np.asarray(inp["z2"], dtype=f32).astype(BF16)
    sz1 = (s1_bf.astype(f32) * z1_bf.astype(f32)).astype(BF16)
    sz3 = (s3_bf.astype(f32) * z3_bf.astype(f32)).astype(BF16)
    sz2 = (s2_bf.astype(f32) * z2_bf.astype(f32)).astype(BF16)

    def szp(a, b):
        z = np.zeros((64, H), dtype=BF16)
        z[0:GL] = a
        z[32:32 + GL] = b
        return z.T  # (4096, 64)

    jg_i, q_i = np.meshgrid(np.arange(7), np.arange(Q), indexing="ij")
    gl_i, q2_i = np.meshgrid(np.arange(GL), np.arange(Q), indexing="ij")

    def sx(sbf):
        # sbf: (28, 4096) local-group scales. Layout per partition p:
        # half A: [v:8][i:4][m=4*dk+jg:16][d:2]; half B (+1024): njg=3.
        out = np.zeros((128, NSH), dtype=BF16)
        for half, (njg, jg0, base) in enumerate(((4, 0, 0), (3, 4, 1024))):
            v, i, m, dd, p = np.meshgrid(
                np.arange(NV), np.arange(4), np.arange(KC * njg), np.arange(2),
                np.arange(128), indexing="ij")
            dk = m // njg
            jg = m % njg
            g = 4 * (jg0 + jg) + i
            k = (v * KC + dk) * 128 + p
            idx = base + v * (4 * KC * njg * 2) + i * (njg * KC * 2) + m * 2 + dd
            out[p.reshape(-1), idx.reshape(-1)] = sbf[g.reshape(-1), k.reshape(-1)]
        return out

    # s2 column permutation: newcol i*16+l <- oldcol 4l+i
    i_, l_ = np.meshgrid(np.arange(4), np.arange(16), indexing="ij")
    s2perm = (4 * l_ + i_).reshape(-1)

    n_loc = np.arange(NSH)
    ind1g = np.where(n_loc[None, :] // Q == np.arange(GL)[:, None],
                     -1.0, 0.0).astype(BF16)
    ind1 = np.zeros((64, NSH), dtype=BF16)
    ind1[0:GL] = ind1g
    ind1[32:32 + GL] = ind1g
    i_c, c_c = np.meshgrid(np.arange(4), np.arange(128), indexing="ij")
    g2_of = ((4 * c_c + i_c) % 64).reshape(-1)      # (512,) for col i*128+c
    ind2 = np.where(g2_of[None, :] == np.arange(G2)[:, None],
                    -1.0, 0.0).astype(BF16)

    in_maps = []
    for r in range(NCORES):
        rows = (56 * q_i + 7 * r + jg_i).reshape(-1)        # j = jg*64+q
        fcols = (224 * q2_i + 28 * r + gl_i).reshape(-1)    # n_loc = gl*64+q
        gs = slice(GL * r, GL * (r + 1))
        in_maps.append({
            "xT": xT,
            "pk1": np.ascontiguousarray(qw1_u[rows].T),     # (4096, 448)
            "pk3": np.ascontiguousarray(qw3_u[rows].T),
            "pk2": np.ascontiguousarray(qw2_u[:, fcols].T),  # (1792, 1024)
            "s1x": sx(s1_bf[gs]),
            "s3x": sx(s3_bf[gs]),
            "szT": np.ascontiguousarray(szp(sz1[gs], sz3[gs])),
            "s2d": np.ascontiguousarray(s2_bf[:, fcols].T[:, s2perm]),
            "sz2T": np.ascontiguousarray(sz2[:, fcols].T),
            "ind1": ind1,
            "ind2": ind2,
        })
    return in_maps


def kernel(x, qw1, s1, z1, qw3, s3, z3, qw2, s2, z2, groupsize=64, **_ignored):
    from concourse.bass_utils import run_bass_kernel_spmd

    global LAST_EXEC_NS

    in_maps = build_in_maps(dict(x=x, qw1=qw1, s1=s1, z1=z1, qw3=qw3, s3=s3,
                                 z3=z3, qw2=qw2, s2=s2, z2=z2))
    _cache["in_maps"] = in_maps

    nc = _get_nc()
    trace = bool(os.environ.get("BASS_HQQ_TRACE"))
    try:
        res = run_bass_kernel_spmd(nc, in_maps, list(range(NCORES)), trace=trace)
    except ModuleNotFoundError:
        res = run_bass_kernel_spmd(nc, in_maps, list(range(NCORES)), trace=False)
    LAST_EXEC_NS = res.exec_time_ns

    acc = np.zeros((H, M), dtype=np.float32)
    for r in range(NCORES):
        acc += np.asarray(res.results[r]["out"], dtype=np.float32)
    hidp = np.arange(H)
    hid = 4 * (hidp % 1024) + hidp // 1024
    out = np.empty((H, M), dtype=np.float32)
    out[hid] = acc
    return out.T.astype(np.float32)
